# revision 1
# baseline (speedup 1.0000x reference)
"""DeepSeek-V3 MoE gate (nn_MoEGate) Trainium2 Bass kernel.

Math (per token): logits = x @ w; s = sigmoid(logits) + bias;
hierarchical top-k: per-group top-2 sums -> top-4 groups -> mask ->
top-8 experts; weights = normalized masked scores * 2.5.

Implementation notes:
- Token-parallel across 8 cores (2048 tokens each).
- Matmul in 3-pass fp16 hi/lo compensation: x, w split into fp16 hi+lo
  (scaled by 2^8 / 2^11 to keep lo parts in fp16 normal range); computes
  hi*hi + hi*lo + lo*hi for ~fp32 accuracy at bf16 speed (1 cyc/row).
- sigmoid computed as 0.5*(1+tanh(x/2)) -- ACT's tanh table is 4-ULP vs
  sigmoid's 40-ULP. The 0.5 scale is folded into the weight scaling, and
  everything downstream works with s2 = 2*(sigmoid+bias), which leaves
  the top-k selection and the normalized weights invariant.
- PE computes scoresT [e, tok] (weight-stationary); PE-transposes to
  [tok, e] for the per-token top-k on the vector engine, which uses the
  hardware top-8 (Max8/MaxIndex) instructions.
"""
import numpy as np

import concourse.bass as bass
import concourse.mybir as mybir
import concourse.tile as tile
from concourse.bass_utils import run_bass_kernel_spmd
from concourse.masks import make_identity

F32 = mybir.dt.float32
F16 = mybir.dt.float16
U32 = mybir.dt.uint32

N_CORES = 8
BSZ, SEQ, H = 4, 4096, 7168
N_TOK = BSZ * SEQ                  # 16384
TPC = N_TOK // N_CORES             # 2048 tokens per core
E = 256                            # experts
G, EPG = 8, 32                     # groups, experts/group
CHUNK = 512
N_CHUNKS = TPC // CHUNK            # 4
KQ, KJ = 14, 4                     # 56 k-tiles as 14 quads of 4
KT = KQ * KJ

SCALE_X = 2.0 ** 8
SCALE_W = 2.0 ** 11                # w * 0.5 * 2^12
ACT_SCALE = 2.0 ** -20             # undo SCALE_X*SCALE_W -> x.w*0.5
ROUTED_SCALING = 2.5


def _split_caps(nc):
    """Split >1-wait sync_info into standalone EventSemaphore insts.

    This walrus build accepts at most one sem wait per engine
    instruction (EventSemaphore holds two)."""
    n = 0
    for fn in nc.m.functions:
        for bb in fn.blocks:
            insts = bb.instructions
            new = []
            changed = False
            for inst in insts:
                si = inst.sync_info
                waits = list(si.on_wait) if si is not None and si.on_wait else []
                if len(waits) > 1 and str(inst.opcode) != "EventSemaphore":
                    excess, keep = waits[:-1], waits[-1:]
                    for i in range(0, len(excess), 2):
                        ev = mybir.InstEventSemaphore(
                            name=f"EVW-{inst.name}-{i}", engine=inst.engine
                        )
                        ev.sync_info = mybir.SyncInfo(
                            on_wait=excess[i:i + 2], on_update=[]
                        )
                        new.append(ev)
                        n += 1
                    inst.sync_info = mybir.SyncInfo(
                        on_wait=keep,
                        on_update=list(si.on_update) if si.on_update else [],
                    )
                    changed = True
                new.append(inst)
            if changed:
                insts[:] = new
    return n


def build_nc(n_chunks=N_CHUNKS, repeat=1, mode="full", xbufs=3, tsplit=False):
    nc = bass.Bass("TRN2", target_bir_lowering=False, debug=False)

    # X2: x^T, scaled, fp16 hi/lo, laid out partition-major per k-quad:
    #   [p=128, kq=14, j=4, hl=2, tok=2048]  with h = kq*512 + j*128 + p
    X2 = nc.dram_tensor("X2", [128, KQ, KJ, 2, TPC], F16, kind="ExternalInput").ap()
    # W2: w, scaled, fp16 hi/lo: [p=128, kq, j, hl, e=256]
    W2 = nc.dram_tensor("W2", [128, KQ, KJ, 2, E], F16, kind="ExternalInput").ap()
    # B2: 1 + 2*bias, replicated: [128, 256]
    B2 = nc.dram_tensor("B2", [128, E], F32, kind="ExternalInput").ap()

    OIDX = nc.dram_tensor("OIDX", [TPC, 8], U32, kind="ExternalOutput").ap()
    OWTS = nc.dram_tensor("OWTS", [TPC, 8], F32, kind="ExternalOutput").ap()

    with tile.TileContext(nc) as tc:
        with (
            tc.tile_pool(name="const", bufs=1) as cpool,
            tc.tile_pool(name="xs", bufs=xbufs) as xpool,
            tc.tile_pool(name="pm", bufs=2, space="PSUM") as pmpool,
            tc.tile_pool(name="pt", bufs=3, space="PSUM") as ptpool,
            tc.tile_pool(name="ts", bufs=4) as tpool,
            tc.tile_pool(name="sc", bufs=2) as spool,
            tc.tile_pool(name="sm", bufs=4) as smpool,
            tc.tile_pool(name="out", bufs=1) as opool,
        ):
            w2 = cpool.tile([128, KQ, KJ, 2, E], F16)
            nc.sync.dma_start(w2[:], W2)
            bias2 = cpool.tile([128, E], F32)
            nc.sync.dma_start(bias2[:], B2)
            ident = cpool.tile([128, 128], F32)
            make_identity(nc, ident[:])

            oidx = opool.tile([128, TPC // 128, 8], U32)
            owts = opool.tile([128, TPC // 128, 8], F32)
            if mode != "full":
                nc.gpsimd.memset(oidx[:], 0)
                nc.gpsimd.memset(owts[:], 0.0)

            pms = {}

            def mm_phase(chunk, kqs):
                if chunk not in pms:
                    pms[chunk] = [
                        pmpool.tile([128, CHUNK], F32, tag=f"pm{e}", name=f"pm{e}")
                        for e in (0, 1)
                    ]
                pm = pms[chunk]
                for kq in kqs:
                    xt = xpool.tile([128, KJ, 2, CHUNK], F16)
                    nc.sync.dma_start(
                        xt[:], X2[:, kq, :, :, chunk * CHUNK:(chunk + 1) * CHUNK]
                    )
                    for j in range(KJ):
                        k = kq * KJ + j
                        xh = xt[:, j, 0, :]
                        xl = xt[:, j, 1, :]
                        for e in (0, 1):
                            wh = w2[:, kq, j, 0, e * 128:(e + 1) * 128]
                            wl = w2[:, kq, j, 1, e * 128:(e + 1) * 128]
                            if mode == "mm1":
                                nc.tensor.matmul(
                                    pm[e][:], wh, xh,
                                    start=(k == 0), stop=(k == KT - 1),
                                )
                                continue
                            nc.tensor.matmul(
                                pm[e][:], wh, xh, start=(k == 0), stop=False
                            )
                            nc.tensor.matmul(pm[e][:], wh, xl, start=False, stop=False)
                            nc.tensor.matmul(
                                pm[e][:], wl, xh, start=False, stop=(k == KT - 1)
                            )
            def post_phase(chunk):
                if mode in ("mm_only", "mm1"):
                    return
                pm = pms.pop(chunk)
                # tanh(x.w/2) -> SBUF, [e, tok] layout
                ts = []
                for e in (0, 1):
                    t = tpool.tile([128, CHUNK], F32, tag=f"t{e}")
                    if tsplit:
                        for j in range(CHUNK // 128):
                            nc.scalar.activation(
                                t[:, j * 128:(j + 1) * 128],
                                pm[e][:, j * 128:(j + 1) * 128],
                                mybir.ActivationFunctionType.Tanh,
                                scale=ACT_SCALE,
                            )
                    else:
                        nc.scalar.activation(
                            t[:], pm[e][:], mybir.ActivationFunctionType.Tanh,
                            scale=ACT_SCALE,
                        )
                    ts.append(t)
                # phase A: transpose to [tok, e] and add bias -- kept ahead
                # of the long DVE chains so PE never stalls on DVE slots
                s2s = []
                for j in range(CHUNK // 128):
                    pt = ptpool.tile([128, E], F32)
                    for e in (0, 1):
                        nc.tensor.matmul(
                            pt[:, e * 128:(e + 1) * 128],
                            ts[e][:, j * 128:(j + 1) * 128],
                            ident[:],
                            is_transpose=True,
                            start=(e == 0),
                            stop=(e == 1),
                        )
                    # s2 = 2*sigmoid + 2*bias = tanh + (1 + 2*bias)
                    s2 = spool.tile([128, E], F32, name=f"s2_{j}")
                    nc.vector.tensor_tensor(
                        s2[:], pt[:], bias2[:], op=mybir.AluOpType.add
                    )
                    s2s.append(s2)
                if mode == "half_post":
                    return
                # phase B: per-token hierarchical top-k (pure DVE; overlaps
                # the next chunk's matmuls)
                for j in range(CHUNK // 128):
                    tok0 = chunk * (CHUNK // 128) + j
                    s2 = s2s[j]
                    # group scores: top-2 sum within each group of 32
                    g8 = smpool.tile([128, G, 8], F32, tag="g8")
                    for g in range(G):
                        nc.vector.max(
                            out=g8[:, g, :], in_=s2[:, g * EPG:(g + 1) * EPG]
                        )
                    gs = smpool.tile([128, G], F32, tag="gs")
                    nc.vector.reduce_sum(
                        gs[:], g8[:, :, 0:2], axis=mybir.AxisListType.X
                    )
                    gss = smpool.tile([128, G], F32, tag="gss")
                    nc.vector.max(out=gss[:], in_=gs[:])
                    gmask = smpool.tile([128, G], F32, tag="gmask")
                    nc.vector.tensor_scalar(
                        gmask[:], gs[:], gss[:, 3:4], None,
                        op0=mybir.AluOpType.is_ge,
                    )
                    s2m = spool.tile([128, E], F32, tag="s2m")
                    nc.vector.tensor_tensor(
                        s2m[:].rearrange("p (g e) -> p g e", g=G),
                        s2[:].rearrange("p (g e) -> p g e", g=G),
                        gmask[:].to_broadcast([128, G, EPG]),
                        op=mybir.AluOpType.mult,
                    )
                    # top-8 experts
                    mx = smpool.tile([128, 8], F32, tag="mx")
                    nc.vector.max(out=mx[:], in_=s2m[:])
                    nc.vector.max_index(
                        out=oidx[:, tok0, :], in_max=mx[:], in_values=s2m[:]
                    )
                    # normalize: w = mx / sum(mx) * 2.5
                    sm = smpool.tile([128, 1], F32, tag="sm")
                    nc.vector.reduce_sum(sm[:], mx[:], axis=mybir.AxisListType.X)
                    rc = smpool.tile([128, 1], F32, tag="rc")
                    nc.vector.reciprocal(rc[:], sm[:])
                    nc.vector.tensor_scalar(
                        owts[:, tok0, :], mx[:], rc[:, 0:1], ROUTED_SCALING,
                        op0=mybir.AluOpType.mult, op1=mybir.AluOpType.mult,
                    )

            def trace_all():
                for c in range(n_chunks):
                    mm_phase(c, range(KQ))
                    post_phase(c)

            if repeat == 1:
                trace_all()
            else:
                with tc.For_i(0, repeat, 1):
                    trace_all()

            nc.sync.dma_start(
                OIDX.rearrange("(t p) k -> p t k", p=128), oidx[:]
            )
            nc.sync.dma_start(
                OWTS.rearrange("(t p) k -> p t k", p=128), owts[:]
            )

    _split_caps(nc)
    return nc


def prep_inputs(hidden_states, weight, bias):
    """Host-side: scale, fp16 hi/lo split, transpose, per-core layout."""
    x = np.ascontiguousarray(hidden_states, dtype=np.float32).reshape(N_TOK, H)

    ws = (weight.astype(np.float32) * SCALE_W)          # [H, E]
    ws_hi = ws.astype(np.float16)
    ws_lo = (ws - ws_hi.astype(np.float32)).astype(np.float16)
    # -> [p, kq, j, hl, e]
    w2 = np.stack([ws_hi, ws_lo], axis=1).reshape(KQ, KJ, 128, 2, E)
    w2 = np.ascontiguousarray(w2.transpose(2, 0, 1, 3, 4))

    b2 = (1.0 + 2.0 * bias.astype(np.float32))[None, :]
    b2 = np.ascontiguousarray(np.broadcast_to(b2, (128, E)))

    in_maps = []
    for c in range(N_CORES):
        xc = x[c * TPC:(c + 1) * TPC] * SCALE_X          # [TPC, H] f32
        xh = xc.astype(np.float16)
        xl = (xc - xh.astype(np.float32)).astype(np.float16)
        # [TPC, H] -> [H, TPC] -> [kq, j, p, hl, tok] -> [p, kq, j, hl, tok]
        x2 = np.stack([xh.T, xl.T], axis=1)              # [H, 2, TPC]
        x2 = x2.reshape(KQ, KJ, 128, 2, TPC).transpose(2, 0, 1, 3, 4)
        in_maps.append(
            dict(X2=np.ascontiguousarray(x2), W2=w2, B2=b2)
        )
    return in_maps


_NC_CACHE = {}


def kernel(hidden_states, weight, bias):
    key = "main"
    if key not in _NC_CACHE:
        _NC_CACHE[key] = build_nc()
    nc = _NC_CACHE[key]
    in_maps = prep_inputs(hidden_states, weight, bias)
    res = run_bass_kernel_spmd(nc, in_maps, core_ids=list(range(N_CORES)))
    idx = np.concatenate(
        [r["OIDX"].astype(np.int32) for r in res.results], axis=0
    ).reshape(N_TOK, 8)
    wts = np.concatenate([r["OWTS"] for r in res.results], axis=0).reshape(N_TOK, 8)
    return idx, wts



# revision 3
# speedup vs baseline: 1.3708x; 1.3708x over previous
"""DeepSeek-V3 MoE gate (nn_MoEGate) Trainium2 Bass kernel — v2.

Math (per token): logits = x @ w; s = sigmoid(logits) + bias;
hierarchical top-k: per-group top-2 sums -> top-4 groups -> mask ->
top-8 experts; weights = normalized masked scores * 2.5.

v2 matmul scheme (2.13 pass-units instead of 3):
- main pass: xh(fp16) @ wh(fp16), 1 cyc/col.
- correction pass: one fp8e4 DoubleRow matmul per k-tile computes BOTH
  cross terms in a single 256-deep contraction: pair0 = xl8*wh8,
  pair1 = xh8*wl8 (scales chosen PSUM-neutral: xl*2^4 . wh*2^-4 and
  xh*2^-6 . wl*2^6). Residual error ~10% of 1-pass-fp16 error:
  rel_i ~ 6e-3 (CPU-verified) vs 2e-2 gate.
- xh8 (= xh * 2^-6 in fp8) is converted on-chip by the idle ACT engine,
  so HBM traffic is xh(2B) + xl8(1B) = 3B/elem vs 4B/elem baseline.
- sigmoid via tanh: s2 = 2*(sigmoid+bias) = tanh(x.w/2) + (1+2*bias);
  top-k/normalized weights are invariant under the affine map.
- PE computes scoresT [e, tok]; PE-transposes to [tok, e] for the
  per-token top-k on the vector engine (hardware Max8/MaxIndex).
"""
import numpy as np
import ml_dtypes

import concourse.bass as bass
import concourse.mybir as mybir
import concourse.tile as tile
from concourse.bass_utils import run_bass_kernel_spmd
from concourse.masks import make_identity

F32 = mybir.dt.float32
F16 = mybir.dt.float16
F8 = mybir.dt.float8e4
U32 = mybir.dt.uint32

N_CORES = 8
BSZ, SEQ, H = 4, 4096, 7168
N_TOK = BSZ * SEQ                  # 16384
TPC = N_TOK // N_CORES             # 2048 tokens per core
E = 256                            # experts
G, EPG = 8, 32                     # groups, experts/group
CHUNK = 512
N_CHUNKS = TPC // CHUNK            # 4
KQ, KJ = 14, 4                     # 56 k-tiles as 14 quads of 4
KT = KQ * KJ

SCALE_X = 2.0 ** 8
SCALE_W = 2.0 ** 11                # w * 0.5 * 2^12
ACT_SCALE = 2.0 ** -20             # undo SCALE_X*SCALE_W -> x.w*0.5
ROUTED_SCALING = 2.5
# fp8 encodings (PSUM-scale-neutral pairs)
S_XL, S_WH8 = 2.0 ** 4, 2.0 ** -4    # pair0: (xl*2^4)*(wh*2^-4) = xl*wh
S_XH8, S_WL8 = 2.0 ** -6, 2.0 ** 6   # pair1: (xh*2^-6)*(wl*2^6) = xh*wl
F8_LIM = 240.0


def _split_caps(nc):
    """Split >1-wait sync_info into standalone EventSemaphore insts.

    This walrus build accepts at most one sem wait per engine
    instruction (EventSemaphore holds two)."""
    n = 0
    for fn in nc.m.functions:
        for bb in fn.blocks:
            insts = bb.instructions
            new = []
            changed = False
            for inst in insts:
                si = inst.sync_info
                waits = list(si.on_wait) if si is not None and si.on_wait else []
                if len(waits) > 1 and str(inst.opcode) != "EventSemaphore":
                    excess, keep = waits[:-1], waits[-1:]
                    for i in range(0, len(excess), 2):
                        ev = mybir.InstEventSemaphore(
                            name=f"EVW-{inst.name}-{i}", engine=inst.engine
                        )
                        ev.sync_info = mybir.SyncInfo(
                            on_wait=excess[i:i + 2], on_update=[]
                        )
                        new.append(ev)
                        n += 1
                    inst.sync_info = mybir.SyncInfo(
                        on_wait=keep,
                        on_update=list(si.on_update) if si.on_update else [],
                    )
                    changed = True
                new.append(inst)
            if changed:
                insts[:] = new
    return n


def build_nc(n_chunks=N_CHUNKS, repeat=1, mode="stag", xbufs=None, x8bufs=None,
             ring8=False):
    if xbufs is None:
        xbufs = 6 if mode == "wpair" else 3
    if x8bufs is None:
        x8bufs = 18 if mode == "split" else (6 if mode == "wpair" else 3)
    nc = bass.Bass("TRN2", target_bir_lowering=False, debug=False)

    # X2: xh = fp16(x*2^8), per-chunk contiguous: [p, kq, chunk, j, tok]
    X2 = nc.dram_tensor(
        "X2", [128, KQ, N_CHUNKS, KJ, CHUNK], F16, kind="ExternalInput"
    ).ap()
    # X8L: xl8 = e4m3(xl*2^4): [p, kq, chunk, j, tok]
    X8L = nc.dram_tensor(
        "X8L", [128, KQ, N_CHUNKS, KJ, CHUNK], F8, kind="ExternalInput"
    ).ap()
    # W2: wh fp16: [p, kq, j, e]
    W2 = nc.dram_tensor("W2", [128, KQ, KJ, E], F16, kind="ExternalInput").ap()
    # W8: fp8 stacked pairs [p, kq, j, pair, e]: pair0=wh8, pair1=wl8
    W8 = nc.dram_tensor("W8", [128, KQ, KJ, 2, E], F8, kind="ExternalInput").ap()
    # B2: 1 + 2*bias, replicated: [128, 256]
    B2 = nc.dram_tensor("B2", [128, E], F32, kind="ExternalInput").ap()

    OIDX = nc.dram_tensor("OIDX", [TPC, 8], U32, kind="ExternalOutput").ap()
    OWTS = nc.dram_tensor("OWTS", [TPC, 8], F32, kind="ExternalOutput").ap()

    with tile.TileContext(nc) as tc:
        with (
            tc.tile_pool(name="const", bufs=1) as cpool,
            tc.tile_pool(name="xs", bufs=xbufs) as xpool,
            tc.tile_pool(name="x8", bufs=x8bufs) as x8pool,
            tc.tile_pool(name="pm",
                         bufs=1 if mode in ("wpair", "g1024") else 2,
                         space="PSUM") as pmpool,
            tc.tile_pool(name="pt", bufs=3, space="PSUM") as ptpool,
            tc.tile_pool(name="ts", bufs=4) as tpool,
            tc.tile_pool(name="sc", bufs=2) as spool,
            tc.tile_pool(name="sm", bufs=4) as smpool,
            tc.tile_pool(name="out", bufs=1) as opool,
        ):
            w2 = cpool.tile([128, KQ, KJ, E], F16)
            nc.sync.dma_start(w2[:], W2)
            w8 = cpool.tile([128, KQ, KJ, 2, E], F8)
            nc.sync.dma_start(w8[:], W8)
            bias2 = cpool.tile([128, E], F32)
            nc.sync.dma_start(bias2[:], B2)
            ident = cpool.tile([128, 128], F32)
            make_identity(nc, ident[:])

            oidx = opool.tile([128, TPC // 128, 8], U32)
            owts = opool.tile([128, TPC // 128, 8], F32)
            if mode != "full":
                nc.gpsimd.memset(oidx[:], 0)
                nc.gpsimd.memset(owts[:], 0.0)

            pms = {}

            def mm_phase(chunk, stagger_post=None):
                if chunk not in pms:
                    pms[chunk] = [
                        pmpool.tile([128, CHUNK], F32, tag=f"pm{e}", name=f"pm{e}")
                        for e in (0, 1)
                    ]
                pm = pms[chunk]
                do16 = mode != "dr_only"
                do8 = mode not in ("mm16", "mm16x")
                split = mode == "split"
                x8ts = {}
                for kq in range(KQ):
                    if kq == 1 and stagger_post is not None:
                        # previous chunk's post issues here so its tanh-gated
                        # PE transposes hide under this chunk's kq0 matmuls
                        post_phase(stagger_post)
                    xt = xpool.tile([128, KJ, CHUNK], F16)
                    nc.sync.dma_start(xt[:], X2[:, kq, chunk])
                    if do8:
                        x8t = x8pool.tile([128, KJ, 2, CHUNK], F8)
                        x8ts[kq] = x8t
                        dma8 = nc.scalar.dma_start if ring8 else nc.sync.dma_start
                        dma8(x8t[:, :, 0, :], X8L[:, kq, chunk])
                        # on-chip xh8 = e4m3(xh * 2^-6), split across the
                        # mostly idle ACT and DVE engines
                        nc.scalar.activation(
                            x8t[:, 0:2, 1, :], xt[:, 0:2, :],
                            mybir.ActivationFunctionType.Copy, scale=S_XH8,
                        )
                        nc.vector.tensor_scalar(
                            x8t[:, 2:4, 1, :], xt[:, 2:4, :], S_XH8, None,
                            op0=mybir.AluOpType.mult,
                        )
                    for j in range(KJ):
                        k = kq * KJ + j
                        for e in (0, 1):
                            if do16:
                                wh = w2[:, kq, j, e * 128:(e + 1) * 128]
                                nc.tensor.matmul(
                                    pm[e][:], wh, xt[:, j, :],
                                    start=(k == 0),
                                    stop=(not do8 and k == KT - 1),
                                )
                    if do8 and not split:
                        for j in range(KJ):
                            k = kq * KJ + j
                            for e in (0, 1):
                                w8s = w8[:, kq, j, :, e * 128:(e + 1) * 128]
                                nc.tensor.matmul(
                                    pm[e][:], w8s, x8t[:, j, :, :],
                                    start=(not do16 and k == 0),
                                    stop=(k == KT - 1),
                                    perf_mode=mybir.MatmulPerfMode.DoubleRow,
                                )
                if do8 and split:
                    for kq in range(KQ):
                        for j in range(KJ):
                            k = kq * KJ + j
                            for e in (0, 1):
                                w8s = w8[:, kq, j, :, e * 128:(e + 1) * 128]
                                nc.tensor.matmul(
                                    pm[e][:], w8s, x8ts[kq][:, j, :, :],
                                    start=(not do16 and k == 0),
                                    stop=(k == KT - 1),
                                    perf_mode=mybir.MatmulPerfMode.DoubleRow,
                                )

            def post_phase(chunk):
                if mode in ("mm_only",):
                    return
                pm = pms.pop(chunk)
                # tanh(x.w/2) -> SBUF, [e, tok] layout
                ts = []
                for e in (0, 1):
                    t = tpool.tile([128, CHUNK], F32, tag=f"t{e}")
                    nc.scalar.activation(
                        t[:], pm[e][:], mybir.ActivationFunctionType.Tanh,
                        scale=ACT_SCALE,
                    )
                    ts.append(t)
                # phase A: transpose to [tok, e] and add bias -- kept ahead
                # of the long DVE chains so PE never stalls on DVE slots
                s2s = []
                for j in range(CHUNK // 128):
                    pt = ptpool.tile([128, E], F32)
                    for e in (0, 1):
                        nc.tensor.matmul(
                            pt[:, e * 128:(e + 1) * 128],
                            ts[e][:, j * 128:(j + 1) * 128],
                            ident[:],
                            is_transpose=True,
                            start=(e == 0),
                            stop=(e == 1),
                        )
                    # s2 = 2*sigmoid + 2*bias = tanh + (1 + 2*bias)
                    s2 = spool.tile([128, E], F32, name=f"s2_{j}")
                    nc.vector.tensor_tensor(
                        s2[:], pt[:], bias2[:], op=mybir.AluOpType.add
                    )
                    s2s.append(s2)
                if mode == "half_post":
                    return
                # phase B: per-token hierarchical top-k (pure DVE; overlaps
                # the next chunk's matmuls)
                for j in range(CHUNK // 128):
                    tok0 = chunk * (CHUNK // 128) + j
                    s2 = s2s[j]
                    # group scores: top-2 sum within each group of 32
                    g8 = smpool.tile([128, G, 8], F32, tag="g8")
                    for g in range(G):
                        nc.vector.max(
                            out=g8[:, g, :], in_=s2[:, g * EPG:(g + 1) * EPG]
                        )
                    gs = smpool.tile([128, G], F32, tag="gs")
                    nc.vector.reduce_sum(
                        gs[:], g8[:, :, 0:2], axis=mybir.AxisListType.X
                    )
                    gss = smpool.tile([128, G], F32, tag="gss")
                    nc.vector.max(out=gss[:], in_=gs[:])
                    gmask = smpool.tile([128, G], F32, tag="gmask")
                    nc.vector.tensor_scalar(
                        gmask[:], gs[:], gss[:, 3:4], None,
                        op0=mybir.AluOpType.is_ge,
                    )
                    s2m = spool.tile([128, E], F32, tag="s2m")
                    nc.vector.tensor_tensor(
                        s2m[:].rearrange("p (g e) -> p g e", g=G),
                        s2[:].rearrange("p (g e) -> p g e", g=G),
                        gmask[:].to_broadcast([128, G, EPG]),
                        op=mybir.AluOpType.mult,
                    )
                    # top-8 experts
                    mx = smpool.tile([128, 8], F32, tag="mx")
                    nc.vector.max(out=mx[:], in_=s2m[:])
                    nc.vector.max_index(
                        out=oidx[:, tok0, :], in_max=mx[:], in_values=s2m[:]
                    )
                    # normalize: w = mx / sum(mx) * 2.5
                    sm = smpool.tile([128, 1], F32, tag="sm")
                    nc.vector.reduce_sum(sm[:], mx[:], axis=mybir.AxisListType.X)
                    rc = smpool.tile([128, 1], F32, tag="rc")
                    nc.vector.reciprocal(rc[:], sm[:])
                    nc.vector.tensor_scalar(
                        owts[:, tok0, :], mx[:], rc[:, 0:1], ROUTED_SCALING,
                        op0=mybir.AluOpType.mult, op1=mybir.AluOpType.mult,
                    )

            def mm_phase_pair(c0, c1):
                """Both chunks of a pair stream back-to-back through the same
                stationary weights; a post-pass (_dedupe_ldw) then drops the
                second Ldweights of each pair."""
                for c in (c0, c1):
                    pms[c] = [
                        pmpool.tile(
                            [128, CHUNK], F32,
                            tag=f"pm_{c % 2}_{e}", name=f"pm_{c % 2}_{e}",
                        )
                        for e in (0, 1)
                    ]
                pm0, pm1 = pms[c0], pms[c1]
                for kq in range(KQ):
                    xts, x8ts_ = [], []
                    for ci, c in enumerate((c0, c1)):
                        xt = xpool.tile([128, KJ, CHUNK], F16)
                        nc.sync.dma_start(xt[:], X2[:, kq, c])
                        x8t = x8pool.tile([128, KJ, 2, CHUNK], F8)
                        nc.sync.dma_start(x8t[:, :, 0, :], X8L[:, kq, c])
                        if ci == 0:
                            nc.scalar.activation(
                                x8t[:, :, 1, :], xt[:],
                                mybir.ActivationFunctionType.Copy, scale=S_XH8,
                            )
                        else:
                            nc.vector.tensor_scalar(
                                x8t[:, :, 1, :], xt[:], S_XH8, None,
                                op0=mybir.AluOpType.mult,
                            )
                        xts.append(xt)
                        x8ts_.append(x8t)
                    for j in range(KJ):
                        k = kq * KJ + j
                        for e in (0, 1):
                            wh = w2[:, kq, j, e * 128:(e + 1) * 128]
                            for pm, xt in ((pm0, xts[0]), (pm1, xts[1])):
                                nc.tensor.matmul(
                                    pm[e][:], wh, xt[:, j, :],
                                    start=(k == 0), stop=False,
                                )
                    for j in range(KJ):
                        k = kq * KJ + j
                        for e in (0, 1):
                            w8s = w8[:, kq, j, :, e * 128:(e + 1) * 128]
                            for pm, x8t in ((pm0, x8ts_[0]), (pm1, x8ts_[1])):
                                nc.tensor.matmul(
                                    pm[e][:], w8s, x8t[:, j, :, :],
                                    start=False, stop=(k == KT - 1),
                                    perf_mode=mybir.MatmulPerfMode.DoubleRow,
                                )

            def mm_phase_g(g):
                """1024-token group 2g/2g+1: fp16 matmuls stream N=1024 into
                a 2-bank PSUM tile (half the fp16 MM count + LDW per token);
                DR correction stays at N=512 per half."""
                c0, c1 = 2 * g, 2 * g + 1
                pmg = [
                    pmpool.tile([128, 2, CHUNK], F32, tag=f"pmg{e}",
                                name=f"pmg{e}")
                    for e in (0, 1)
                ]
                pms[c0] = [pmg[e][:, 0, :] for e in (0, 1)]
                pms[c1] = [pmg[e][:, 1, :] for e in (0, 1)]
                for kq in range(KQ):
                    xt = xpool.tile([128, 2, KJ, CHUNK], F16)
                    nc.sync.dma_start(xt[:], X2[:, kq, c0:c1 + 1])
                    x8t = x8pool.tile([128, 2, KJ, 2, CHUNK], F8)
                    dma8 = nc.scalar.dma_start if ring8 else nc.sync.dma_start
                    dma8(x8t[:, :, :, 0, :], X8L[:, kq, c0:c1 + 1])
                    nc.scalar.activation(
                        x8t[:, 0, :, 1, :], xt[:, 0, :, :],
                        mybir.ActivationFunctionType.Copy, scale=S_XH8,
                    )
                    nc.vector.tensor_scalar(
                        x8t[:, 1, :, 1, :], xt[:, 1, :, :], S_XH8, None,
                        op0=mybir.AluOpType.mult,
                    )
                    for j in range(KJ):
                        k = kq * KJ + j
                        for e in (0, 1):
                            wh = w2[:, kq, j, e * 128:(e + 1) * 128]
                            nc.tensor.matmul(
                                pmg[e][:, :, :], wh, xt[:, :, j, :],
                                start=(k == 0), stop=False,
                            )
                    for j in range(KJ):
                        k = kq * KJ + j
                        for e in (0, 1):
                            w8s = w8[:, kq, j, :, e * 128:(e + 1) * 128]
                            for h in (0, 1):
                                nc.tensor.matmul(
                                    pmg[e][:, h, :], w8s, x8t[:, h, j, :, :],
                                    start=False,
                                    stop=(k == KT - 1 and h == 1),
                                    perf_mode=mybir.MatmulPerfMode.DoubleRow,
                                )

            def trace_all():
                if mode == "g1024":
                    for g in range(n_chunks // 2):
                        mm_phase_g(g)
                        post_phase(2 * g)
                        post_phase(2 * g + 1)
                    return
                if mode == "wpair":
                    for cp in range(n_chunks // 2):
                        mm_phase_pair(2 * cp, 2 * cp + 1)
                        post_phase(2 * cp)
                        post_phase(2 * cp + 1)
                    return
                if mode == "stag":
                    for c in range(n_chunks):
                        mm_phase(c, stagger_post=(c - 1 if c > 0 else None))
                    post_phase(n_chunks - 1)
                    return
                for c in range(n_chunks):
                    mm_phase(c)
                    post_phase(c)

            if repeat == 1:
                trace_all()
            else:
                with tc.For_i(0, repeat, 1):
                    trace_all()

            nc.sync.dma_start(
                OIDX.rearrange("(t p) k -> p t k", p=128), oidx[:]
            )
            nc.sync.dma_start(
                OWTS.rearrange("(t p) k -> p t k", p=128), owts[:]
            )

    _split_caps(nc)
    _dedupe_ldw(nc)
    return nc


def _dedupe_ldw(nc):
    """Delete back-to-back redundant PE Ldweights (identical stationary AP
    and mode, only non-transpose Matmults in between on the PE, no sync
    waits/updates on the dupe). The following Matmults then reuse the
    already-loaded weights, halving PE sequencer dispatch load."""
    n = 0
    for fn in nc.m.functions:
        for bb in fn.blocks:
            insts = bb.instructions
            keep = []
            last_key = None
            for inst in insts:
                op = str(inst.opcode)
                eng = str(inst.engine)
                if "PE" not in eng:
                    keep.append(inst)
                    continue
                if op == "Ldweights":
                    si = inst.sync_info
                    waits = list(si.on_wait) if si is not None and si.on_wait else []
                    ups = list(si.on_update) if si is not None and si.on_update else []
                    key = (
                        repr(inst.ins[0]),
                        str(getattr(inst, "perf_mode", None)),
                        str(getattr(inst, "is_transpose", None)),
                        str(getattr(inst, "tile_position", None)),
                    )
                    if key == last_key and not waits and not ups:
                        n += 1
                        continue
                    last_key = key
                elif op == "Matmult":
                    # self-loading transpose matmuls clobber the array weights
                    if getattr(inst, "is_transpose", None):
                        last_key = None
                elif op in ("EventSemaphore", "Drain", "RegisterMove", "Nop"):
                    pass  # no effect on the PE weight buffer
                else:
                    last_key = None
                keep.append(inst)
            if len(keep) != len(insts):
                insts[:] = keep
    return n


def _e4m3(a):
    return np.clip(a, -F8_LIM, F8_LIM).astype(ml_dtypes.float8_e4m3)


def prep_inputs(hidden_states, weight, bias):
    """Host-side: scale, fp16/fp8 splits, transpose, per-core layout."""
    x = np.ascontiguousarray(hidden_states, dtype=np.float32).reshape(N_TOK, H)

    ws = (weight.astype(np.float32) * SCALE_W)          # [H, E]
    ws_hi = ws.astype(np.float16)
    ws_lo = ws - ws_hi.astype(np.float32)
    wh8 = _e4m3(ws_hi.astype(np.float32) * S_WH8)
    wl8 = _e4m3(ws_lo * S_WL8)
    # [H, E] -> [p, kq, j, e]
    w2 = np.ascontiguousarray(
        ws_hi.reshape(KQ, KJ, 128, E).transpose(2, 0, 1, 3)
    )
    # [p, kq, j, pair, e]
    w8 = np.stack([wh8, wl8], axis=1).reshape(KQ, KJ, 128, 2, E)
    w8 = np.ascontiguousarray(w8.transpose(2, 0, 1, 3, 4))

    b2 = (1.0 + 2.0 * bias.astype(np.float32))[None, :]
    b2 = np.ascontiguousarray(np.broadcast_to(b2, (128, E)))

    in_maps = []
    for c in range(N_CORES):
        xc = x[c * TPC:(c + 1) * TPC] * SCALE_X          # [TPC, H] f32
        xh = xc.astype(np.float16)
        xl8 = _e4m3((xc - xh.astype(np.float32)) * S_XL)
        # [TPC, H] -> [H, TPC] -> [kq, j, p, chunk, tok] -> [p, kq, chunk, j, tok]
        x2 = xh.T.reshape(KQ, KJ, 128, N_CHUNKS, CHUNK).transpose(2, 0, 3, 1, 4)
        x8 = xl8.T.reshape(KQ, KJ, 128, N_CHUNKS, CHUNK).transpose(2, 0, 3, 1, 4)
        in_maps.append(
            dict(
                X2=np.ascontiguousarray(x2),
                X8L=np.ascontiguousarray(x8),
                W2=w2, W8=w8, B2=b2,
            )
        )
    return in_maps


_NC_CACHE = {}


def kernel(hidden_states, weight, bias):
    key = "main"
    if key not in _NC_CACHE:
        _NC_CACHE[key] = build_nc()
    nc = _NC_CACHE[key]
    in_maps = prep_inputs(hidden_states, weight, bias)
    res = run_bass_kernel_spmd(nc, in_maps, core_ids=list(range(N_CORES)))
    idx = np.concatenate(
        [r["OIDX"].astype(np.int32) for r in res.results], axis=0
    ).reshape(N_TOK, 8)
    wts = np.concatenate([r["OWTS"] for r in res.results], axis=0).reshape(N_TOK, 8)
    return idx, wts
